# revision 17
# baseline (speedup 1.0000x reference)
"""Trainium2 Bass kernel for nn_BGCEncoder (transformer encoder block).

Data-parallel over batch: 16 batch elements / 8 cores = 2 per core.
Activations are feature-major [feat, tokens] on-chip so every matmul
contracts over the partition dim with zero on-device transposes.
All matmul operands are fp16 (fp32 PSUM accumulation); measured
end-to-end relative error ~1e-3.

Structure (per core, T = 2048 tokens):
  A:  x = gelu(WeT.T @ pros_T + be)                  [D, T] fp16
  B:  btl = Wb_s.T @ gelu(WgT.T @ struct_T + bg)     (beta folded into Wb;
      emitted ONLY when beta != 0 — for this model beta == 0 so the whole
      structure branch vanishes and btl == 0)
  V:  v3[t, h, 0:64] = x-as-lhsT @ WvT + bv ; v3[t, h, 64] = 1  (ones col
      gives the softmax denominator through the ctx matmul)
  C+D fused per (head-pair hp, batch b):
      q' = rope(Wq[hp] @ x) (+btl) ; k = rope(Wk[hp] @ x)  [128, 1024]
      per qt chunk (512): per kt block (128): pair-scores psum [128,1024]
      (two K=64 matmuls at row bases 0/64, concurrent), one Exp -> fp16,
      two ctx matmuls accumulate [65, 512] psums (row 64 = denominator).
      ctx + denom copied out unnormalized (DVE).
  Post-D: one batched reciprocal over all 32 denominator rows, then
      per row: K=1 broadcast matmul + DVE multiply to normalize ctx.
  Wo + residual + LN1 ; FFN (gelu) ; + residual LN2  (LN stats via
  ones-matmuls; rstd = Exp(-0.5*Ln(var+eps)) on ACT; row broadcasts via
  K=1 matmuls at base-0)
"""

import os
import numpy as np

B, S, E, G, D, H = 16, 1024, 1280, 3072, 512, 8
HD = D // H            # 64
EPS = 1e-5
N_CORES = 8
B_LOC = B // N_CORES   # 2
T = B_LOC * S          # 2048
KE, KG, KD = E // 128, G // 128, D // 128   # 10, 24, 4
DF = 2 * D             # 1024
KF = DF // 128         # 8
TC = 512               # token chunk (tail phases, attention qt)
NT = T // TC           # 4
TB = 1024              # big token chunk (projection phases)
NTB_BIG = T // TB      # 2
NTB = T // 128         # 16 token blocks (for v)

_BOFF = {}
_off = 0
for _name, _n in [("be", KD), ("bg", KD), ("bq", KD), ("bk", KD), ("bbt", KD),
                  ("bo", KD), ("b1", KF), ("b2", KD), ("g1", KD), ("bn1", KD),
                  ("g2", KD), ("bn2", KD)]:
    _BOFF[_name] = _off
    _off += _n
NBIAS = _off

LAST_RESULT = {}


def _build_module(sim_gelu=False, with_beta=True):
    import concourse.bass as bass
    from concourse import bacc
    import concourse.mybir as mybir
    from concourse.tile import TileContext

    F32 = mybir.dt.float32
    F16 = mybir.dt.float16
    AF = mybir.ActivationFunctionType
    GELU = AF.Sigmoid if sim_gelu else AF.Gelu
    MUL = mybir.AluOpType.mult
    ADD = mybir.AluOpType.add
    SUB = mybir.AluOpType.subtract

    nc = bacc.Bacc("TRN2", target_bir_lowering=False)

    # ---- DRAM tensors ----
    pros_d = nc.dram_tensor("pros_t", [KE, 128, T], F16, kind="ExternalInput")
    wet_d = nc.dram_tensor("wet", [KE, 128, D], F16, kind="ExternalInput")
    if with_beta:
        struct_d = nc.dram_tensor("struct_t", [KG, 128, T], F16, kind="ExternalInput")
        wgt_d = nc.dram_tensor("wgt", [KG, 128, D], F16, kind="ExternalInput")
        wbt_d = nc.dram_tensor("wbt", [KD, 128, D], F16, kind="ExternalInput")
    wqt_d = nc.dram_tensor("wqt", [KD, 128, D], F16, kind="ExternalInput")
    wkt_d = nc.dram_tensor("wkt", [KD, 128, D], F16, kind="ExternalInput")
    wvt_d = nc.dram_tensor("wvt", [KD, 128, D], F16, kind="ExternalInput")
    wot_d = nc.dram_tensor("wot", [KD, 128, D], F16, kind="ExternalInput")
    w1t_d = nc.dram_tensor("w1t", [KD, 128, DF], F16, kind="ExternalInput")
    w2t_d = nc.dram_tensor("w2t", [KF, 128, D], F16, kind="ExternalInput")
    bias_d = nc.dram_tensor("bias_cols", [128, NBIAS], F32, kind="ExternalInput")
    bv_d = nc.dram_tensor("bv_row", [1, D], F32, kind="ExternalInput")
    cos_d = nc.dram_tensor("cos_t", [128, S], F16, kind="ExternalInput")
    sin_d = nc.dram_tensor("sin_t", [128, S], F16, kind="ExternalInput")
    r128_d = nc.dram_tensor("r128t", [128, 128], F16, kind="ExternalInput")
    ones_d = nc.dram_tensor("ones_t", [128, 128], F16, kind="ExternalInput")
    out_d = nc.dram_tensor("out_t", [KD, 128, T], F32, kind="ExternalOutput")

    with TileContext(nc) as tc, nc.allow_low_precision(
            reason="fp16 matmul operands by design; fp32 accumulation in PSUM"):
        with (
            tc.tile_pool(name="const", bufs=1) as constp,
            tc.tile_pool(name="big", bufs=4) as bigp,
        ):
            # ---- constants ----
            bias_sb = constp.tile([128, NBIAS], F32, tag="bias")
            nc.sync.dma_start(out=bias_sb, in_=bias_d.ap())
            cos_sb = constp.tile([128, S], F16, tag="cos")
            sin_sb = constp.tile([128, S], F16, tag="sin")
            nc.sync.dma_start(out=cos_sb, in_=cos_d.ap())
            nc.sync.dma_start(out=sin_sb, in_=sin_d.ap())
            r128_sb = constp.tile([128, 128], F16, tag="r128")
            nc.sync.dma_start(out=r128_sb, in_=r128_d.ap())
            bv_bc = constp.tile([128, D], F32, tag="bvbc")
            nc.gpsimd.dma_start(out=bv_bc, in_=bv_d.ap()[0:1, :].to_broadcast((128, D)))
            ones_col = constp.tile([128, 1], F16, tag="ones_col")
            nc.sync.dma_start(out=ones_col, in_=ones_d.ap()[:, 0:1])
            ones128 = constp.tile([128, 128], F16, tag="ones128")
            nc.sync.dma_start(out=ones128, in_=ones_d.ap())
            eps_sb = constp.tile([128, 1], F32, tag="eps")
            nc.vector.memset(eps_sb, EPS)

            def bcol(name, blk):
                o = _BOFF[name] + blk
                return bias_sb[:, o:o + 1]

            x_sb = bigp.tile([128, KD, T], F16, tag="slab", name="x")

            # ============ phase A: x = gelu(We @ pros + be) ============
            with (
                tc.tile_pool(name="pha", bufs=2) as pha,
                tc.tile_pool(name="phaw", bufs=1) as phaw,
                tc.tile_pool(name="psA", bufs=4, space="PSUM") as psA,
            ):
                wet_sb = phaw.tile([128, KE, D], F16, tag="wet")
                nc.sync.dma_start(out=wet_sb, in_=wet_d.ap().rearrange("k p d -> p k d"))
                for i in range(NT):
                    ts = slice(i * TC, (i + 1) * TC)
                    ps = [psA.tile([128, TC], F32, tag="mm", name=f"psa{_k}")
                          for _k in range(KD)]
                    for kc in range(2):
                        pr = pha.tile([128, 5, TC], F16, tag="pros")
                        nc.sync.dma_start(
                            out=pr,
                            in_=pros_d.ap()[kc * 5:(kc + 1) * 5, :, ts]
                            .rearrange("k p t -> p k t"))
                        for kd in range(KD):
                            for k5 in range(5):
                                k = kc * 5 + k5
                                nc.tensor.matmul(
                                    ps[kd],
                                    wet_sb[:, k, kd * 128:(kd + 1) * 128],
                                    pr[:, k5, :],
                                    start=(k == 0), stop=(k == KE - 1))
                    for kd in range(KD):
                        nc.scalar.activation(
                            out=x_sb[:, kd, ts], in_=ps[kd],
                            func=GELU, bias=bcol("be", kd), scale=1.0)

            # ============ phase B (only when beta != 0) ============
            btl_sb = None
            if with_beta:
                btl_sb = bigp.tile([128, KD, T], F16, tag="slab", name="btl")
                with (
                    tc.tile_pool(name="phb", bufs=2) as phb,
                    tc.tile_pool(name="phbw", bufs=1) as phbw,
                    tc.tile_pool(name="psB", bufs=4, space="PSUM") as psB,
                ):
                    wgt_sb = phbw.tile([128, KG, D], F16, tag="wgt")
                    nc.sync.dma_start(out=wgt_sb,
                                      in_=wgt_d.ap().rearrange("k p d -> p k d"))
                    wbt_sb = phbw.tile([128, KD, D], F16, tag="wbt")
                    nc.sync.dma_start(out=wbt_sb,
                                      in_=wbt_d.ap().rearrange("k p d -> p k d"))
                    for i in range(NT):
                        ts = slice(i * TC, (i + 1) * TC)
                        ps = [psB.tile([128, TC], F32, tag="mm", name=f"psb{_k}")
                              for _k in range(KD)]
                        for kc in range(4):
                            sc = phb.tile([128, 6, TC], F16, tag="struct")
                            nc.sync.dma_start(
                                out=sc,
                                in_=struct_d.ap()[kc * 6:(kc + 1) * 6, :, ts]
                                .rearrange("k p t -> p k t"))
                            for kd in range(KD):
                                for k6 in range(6):
                                    k = kc * 6 + k6
                                    nc.tensor.matmul(
                                        ps[kd],
                                        wgt_sb[:, k, kd * 128:(kd + 1) * 128],
                                        sc[:, k6, :],
                                        start=(k == 0), stop=(k == KG - 1))
                        stc = phb.tile([128, KD, TC], F16, tag="st")
                        for kd in range(KD):
                            nc.scalar.activation(
                                out=stc[:, kd, :], in_=ps[kd],
                                func=GELU, bias=bcol("bg", kd), scale=1.0)
                        for kd in range(KD):
                            pb = psB.tile([128, TC], F32, tag="mm")
                            for k in range(KD):
                                nc.tensor.matmul(
                                    pb, wbt_sb[:, k, kd * 128:(kd + 1) * 128],
                                    stc[:, k, :],
                                    start=(k == 0), stop=(k == KD - 1))
                            nc.scalar.activation(
                                out=btl_sb[:, kd, ts], in_=pb,
                                func=AF.Identity, bias=bcol("bbt", kd), scale=1.0)

            # ============ phase V: v3 (token-major v + ones column) ============
            with (
                tc.tile_pool(name="v3pool", bufs=1) as v3p,
            ):
                v3_sb = v3p.tile([128, NTB, H, HD + 1], F16, tag="v3")
                nc.sync.dma_start(
                    out=v3_sb[:, :, :, HD:HD + 1],
                    in_=ones_d.ap().rearrange("p (a b) -> p a b", b=8)[:, :, :, None])
                with (
                    tc.tile_pool(name="phvw", bufs=1) as phvw,
                    tc.tile_pool(name="psVp", bufs=4, space="PSUM") as psVp,
                ):
                    wv_sb = phvw.tile([128, KD, D], F16, tag="wv")
                    nc.sync.dma_start(out=wv_sb,
                                      in_=wvt_d.ap().rearrange("k p d -> p k d"))
                    for tb in range(NTB):
                        pv = psVp.tile([128, D], F32, tag="mm")
                        for k in range(KD):
                            nc.tensor.matmul(
                                pv, x_sb[:, k, tb * 128:(tb + 1) * 128],
                                wv_sb[:, k, :],
                                start=(k == 0), stop=(k == KD - 1))
                        nc.vector.tensor_tensor(
                            v3_sb[:, tb, :, 0:HD], pv, bv_bc, ADD)

                # ======== fused C+D: per head pair, per batch ========
                qp_sb = bigp.tile([128, KD, T], F16, tag="slab", name="qp")
                kr_sb = bigp.tile([128, KD, T], F16, tag="slab", name="kr")
                ctx_sb = bigp.tile([128, KD, T], F16, tag="slab", name="ctx")
                with (
                    tc.tile_pool(name="phc", bufs=3) as phc,
                    tc.tile_pool(name="phcw", bufs=1) as phcw,
                    tc.tile_pool(name="phd", bufs=4) as phd,
                    tc.tile_pool(name="dnp", bufs=2) as dnp,
                    tc.tile_pool(name="psC", bufs=1, space="PSUM") as psC,
                    tc.tile_pool(name="psS", bufs=2, space="PSUM") as psS,
                    tc.tile_pool(name="psX", bufs=2, space="PSUM") as psX,
                ):
                    wq_sb = phcw.tile([128, KD, D], F16, tag="wq")
                    nc.sync.dma_start(out=wq_sb,
                                      in_=wqt_d.ap().rearrange("k p d -> p k d"))
                    wk_sb = phcw.tile([128, KD, D], F16, tag="wk")
                    nc.sync.dma_start(out=wk_sb,
                                      in_=wkt_d.ap().rearrange("k p d -> p k d"))

                    def proj_rope(w_sb, bname, dst, add_btl, hp, i):
                        # one batch (TB=1024 tokens): matmuls in 512 halves,
                        # DVE rope over the full 1024-wide tiles
                        qt = phc.tile([128, TB], F16, tag="qtmp")
                        for half in range(2):
                            hs = slice(i * TB + half * TC, i * TB + (half + 1) * TC)
                            pq = psC.tile([128, TC], F32, tag="pq")
                            for k in range(KD):
                                nc.tensor.matmul(
                                    pq, w_sb[:, k, hp * 128:(hp + 1) * 128],
                                    x_sb[:, k, hs],
                                    start=(k == 0), stop=(k == KD - 1))
                            nc.vector.tensor_scalar(
                                out=qt[:, half * TC:(half + 1) * TC], in0=pq,
                                scalar1=bcol(bname, hp), scalar2=None, op0=ADD)
                        t2 = phc.tile([128, TB], F16, tag="rt2")
                        for half in range(2):
                            prot = psC.tile([128, TC], F32, tag="pq")
                            nc.tensor.matmul(prot, r128_sb,
                                             qt[:, half * TC:(half + 1) * TC],
                                             start=True, stop=True)
                            nc.vector.tensor_tensor(
                                t2[:, half * TC:(half + 1) * TC], prot,
                                sin_sb[:, half * TC:(half + 1) * TC], MUL)
                        t1 = phc.tile([128, TB], F16, tag="rt1")
                        nc.vector.tensor_tensor(t1, qt, cos_sb, MUL)
                        ts = slice(i * TB, (i + 1) * TB)
                        dslice = dst[:, hp, ts]
                        if add_btl:
                            nc.vector.tensor_tensor(t1, t1, t2, ADD)
                            nc.vector.tensor_tensor(
                                dslice, t1, btl_sb[:, hp, ts], ADD)
                        else:
                            nc.vector.tensor_tensor(dslice, t1, t2, ADD)

                    scale = float(1.0 / np.sqrt(HD))
                    NQ = S // TC   # qt chunks per batch (2)
                    NJ = S // 128  # kt blocks per batch (8)
                    for hp in range(KD):
                        for b in range(B_LOC):
                            proj_rope(wq_sb, "bq", qp_sb, with_beta, hp, b)
                            proj_rope(wk_sb, "bk", kr_sb, False, hp, b)
                            # 4 denominator rows (qi x hh) packed at legal bases
                            dn_pack = dnp.tile([128, TC], F32, tag="dn")
                            nc.vector.memset(dn_pack, 1.0)
                            for qi in range(NQ):
                                qcol = b * S + qi * TC
                                c0 = psX.tile([HD + 1, TC], F32, tag="ctx", name="c0")
                                c1 = psX.tile([HD + 1, TC], F32, tag="ctx", name="c1")
                                cpair = (c0, c1)
                                for j in range(NJ):
                                    kcol = b * S + j * 128
                                    sp = psS.tile([128, TB], F32, tag="sc")
                                    for hh in range(2):
                                        r0 = hh * 64
                                        nc.tensor.matmul(
                                            sp[:, hh * TC:(hh + 1) * TC],
                                            kr_sb[r0:r0 + 64, hp, kcol:kcol + 128],
                                            qp_sb[r0:r0 + 64, hp, qcol:qcol + TC],
                                            start=True, stop=True)
                                    ee = phd.tile([128, TB], F16, tag="exp")
                                    nc.scalar.activation(out=ee, in_=sp, func=AF.Exp,
                                                         scale=scale)
                                    for hh in range(2):
                                        nc.tensor.matmul(
                                            cpair[hh],
                                            v3_sb[:, b * 8 + j, hp * 2 + hh, :],
                                            ee[:, hh * TC:(hh + 1) * TC],
                                            start=(j == 0), stop=(j == NJ - 1))
                                for hh in range(2):
                                    r0 = hh * 64
                                    base = 32 * (qi * 2 + hh)
                                    nc.vector.tensor_copy(
                                        out=ctx_sb[r0:r0 + 64, hp, qcol:qcol + TC],
                                        in_=cpair[hh][0:HD, :])
                                    nc.vector.tensor_copy(
                                        out=dn_pack[base:base + 1, :],
                                        in_=cpair[hh][HD:HD + 1, :])
                            # one reciprocal covers the 4 rows; rows already
                            # sit at legal bases for the K=1 broadcast matmul
                            dinv_pk = dnp.tile([128, TC], F16, tag="dinv")
                            nc.vector.reciprocal(out=dinv_pk, in_=dn_pack)
                            for qi in range(NQ):
                                qcol = b * S + qi * TC
                                for hh in range(2):
                                    r0 = hh * 64
                                    base = 32 * (qi * 2 + hh)
                                    pbc = psS.tile([128, TB], F32, tag="sc")
                                    nc.tensor.matmul(
                                        pbc[0:64, 0:TC],
                                        ones128[base:base + 1, 0:64],
                                        dinv_pk[base:base + 1, :],
                                        start=True, stop=True,
                                        tile_position=(base, 0))
                                    cslice = ctx_sb[r0:r0 + 64, hp, qcol:qcol + TC]
                                    nc.vector.tensor_tensor(
                                        cslice, cslice, pbc[0:64, 0:TC], MUL)

            # ============ Wo + residual + LN1 ============
            h_sb = bigp.tile([128, KD, T], F16, tag="slab", name="h")

            def layernorm(i, z, gname, bname, dst, lnp, psbcln):
                ps1 = psbcln.tile([1, TC], F32, tag="s1")
                ps2 = psbcln.tile([1, TC], F32, tag="s2")
                sq = lnp.tile([128, KD, TC], F16, tag="sq")
                for kd in range(KD):
                    nc.vector.tensor_tensor(sq[:, kd, :], z[:, kd, :], z[:, kd, :], MUL)
                for kd in range(KD):
                    nc.tensor.matmul(ps1, ones_col, z[:, kd, :],
                                     start=(kd == 0), stop=(kd == KD - 1))
                for kd in range(KD):
                    nc.tensor.matmul(ps2, ones_col, sq[:, kd, :],
                                     start=(kd == 0), stop=(kd == KD - 1))
                mrow = lnp.tile([1, TC], F32, tag="mrow")
                nc.vector.tensor_scalar_mul(mrow, ps1, 1.0 / D)
                vrow = lnp.tile([1, TC], F32, tag="vrow")
                nc.vector.tensor_scalar_mul(vrow, ps2, 1.0 / D)
                m2 = lnp.tile([1, TC], F32, tag="m2row")
                nc.vector.tensor_tensor(m2, mrow, mrow, MUL)
                nc.vector.tensor_tensor(vrow, vrow, m2, SUB)
                # rstd = exp(-0.5 * ln(var + eps)) on ACT (avoids slow DVE recip)
                lrow = lnp.tile([1, TC], F32, tag="lrow")
                nc.scalar.activation(out=lrow, in_=vrow, func=AF.Ln,
                                     bias=eps_sb[0:1, :], scale=1.0)
                rstd = lnp.tile([1, TC], F16, tag="rstd")
                nc.scalar.activation(out=rstd, in_=lrow, func=AF.Exp, scale=-0.5)
                sh = lnp.tile([1, TC], F16, tag="shrow")
                nc.vector.tensor_tensor(sh, mrow, rstd, MUL)
                nc.vector.tensor_scalar_mul(sh, sh, -1.0)
                psc = psbcln.tile([128, TC], F32, tag="scbc")
                nc.tensor.matmul(psc, ones128[0:1, :], rstd, start=True, stop=True)
                psh = psbcln.tile([128, TC], F32, tag="shbc")
                nc.tensor.matmul(psh, ones128[0:1, :], sh, start=True, stop=True)
                for kd in range(KD):
                    u = lnp.tile([128, TC], F32, tag="u")
                    nc.vector.tensor_tensor(u, z[:, kd, :], psc, MUL)
                    nc.vector.tensor_tensor(u, u, psh, ADD)
                    nc.scalar.activation(
                        out=dst[:, kd, :] if dst.shape[-1] == TC
                        else dst[:, kd, i * TC:(i + 1) * TC],
                        in_=u, func=AF.Identity,
                        bias=bcol(bname, kd), scale=bcol(gname, kd))

            with (
                tc.tile_pool(name="lnp", bufs=3) as lnp,
                tc.tile_pool(name="phow", bufs=1) as phow,
                tc.tile_pool(name="psO", bufs=2, space="PSUM") as psO,
                tc.tile_pool(name="psbcln", bufs=1, space="PSUM") as psbcln,
            ):
                wot_sb = phow.tile([128, KD, D], F16, tag="wot")
                nc.sync.dma_start(out=wot_sb, in_=wot_d.ap().rearrange("k p d -> p k d"))
                for i in range(NT):
                    ts = slice(i * TC, (i + 1) * TC)
                    z = lnp.tile([128, KD, TC], F16, tag="z")
                    for kd in range(KD):
                        po = psO.tile([128, TC], F32, tag="mm")
                        for k in range(KD):
                            nc.tensor.matmul(
                                po, wot_sb[:, k, kd * 128:(kd + 1) * 128],
                                ctx_sb[:, k, ts],
                                start=(k == 0), stop=(k == KD - 1))
                        za = lnp.tile([128, TC], F32, tag="za")
                        nc.scalar.activation(out=za, in_=po, func=AF.Identity,
                                             bias=bcol("bo", kd), scale=1.0)
                        nc.vector.tensor_tensor(z[:, kd, :], za, x_sb[:, kd, ts], ADD)
                    layernorm(i, z, "g1", "bn1", h_sb, lnp, psbcln)

            # ============ FFN + LN2 ============
            ff1a = bigp.tile([128, KD, T], F16, tag="slab", name="ff1a")
            ff1b = bigp.tile([128, KD, T], F16, tag="slab", name="ff1b")
            with (
                tc.tile_pool(name="lnp2", bufs=3) as lnp2,
                tc.tile_pool(name="phfw", bufs=1) as phfw,
                tc.tile_pool(name="outp", bufs=2) as outp,
            ):
                w1_sb = phfw.tile([128, KD, DF], F16, tag="w1")
                nc.sync.dma_start(out=w1_sb, in_=w1t_d.ap().rearrange("k p d -> p k d"))
                w2_sb = phfw.tile([128, KF, D], F16, tag="w2")
                nc.sync.dma_start(out=w2_sb, in_=w2t_d.ap().rearrange("k p d -> p k d"))
                with tc.tile_pool(name="psF1", bufs=3, space="PSUM") as psF1:
                    for i in range(NT):
                        ts = slice(i * TC, (i + 1) * TC)
                        for kf in range(KF):
                            pf = psF1.tile([128, TC], F32, tag="mm")
                            for k in range(KD):
                                nc.tensor.matmul(
                                    pf, w1_sb[:, k, kf * 128:(kf + 1) * 128],
                                    h_sb[:, k, ts],
                                    start=(k == 0), stop=(k == KD - 1))
                            dstf = ff1a if kf < KD else ff1b
                            nc.scalar.activation(
                                out=dstf[:, kf % KD, ts], in_=pf,
                                func=GELU, bias=bcol("b1", kf), scale=1.0)
                with (
                    tc.tile_pool(name="psF2", bufs=2, space="PSUM") as psF2,
                    tc.tile_pool(name="psbcln2", bufs=1, space="PSUM") as psbcln2,
                ):
                  for i in range(NT):
                    ts = slice(i * TC, (i + 1) * TC)
                    z2 = lnp2.tile([128, KD, TC], F16, tag="z")
                    for kd in range(KD):
                        p2 = psF2.tile([128, TC], F32, tag="mm2")
                        for k in range(KF):
                            src = ff1a if k < KD else ff1b
                            nc.tensor.matmul(
                                p2, w2_sb[:, k, kd * 128:(kd + 1) * 128],
                                src[:, k % KD, ts],
                                start=(k == 0), stop=(k == KF - 1))
                        za = lnp2.tile([128, TC], F32, tag="za")
                        nc.scalar.activation(out=za, in_=p2, func=AF.Identity,
                                             bias=bcol("b2", kd), scale=1.0)
                        nc.vector.tensor_tensor(z2[:, kd, :], za, h_sb[:, kd, ts], ADD)
                    oc = outp.tile([128, KD, TC], F32, tag="oc")
                    layernorm(i, z2, "g2", "bn2", oc, lnp2, psbcln2)
                    for kd in range(KD):
                        nc.sync.dma_start(out=out_d.ap()[kd, :, ts], in_=oc[:, kd, :])

    nc.finalize()
    return nc


def _prep_inputs(inputs, with_beta=True):
    f32 = np.float32
    f16 = np.float16

    def col4(vec, nblk):
        return np.ascontiguousarray(np.asarray(vec, f32).reshape(nblk, 128).T)

    beta_cols = np.repeat(np.asarray(inputs['beta'], f32), HD)  # [D]

    bias_cols = np.zeros((128, NBIAS), f32)
    def put(name, vec, nblk):
        bias_cols[:, _BOFF[name]:_BOFF[name] + nblk] = col4(vec, nblk)
    put("be", inputs['be'], KD)
    put("bg", inputs['bg'], KD)
    put("bq", inputs['bq'], KD)
    put("bk", inputs['bk'], KD)
    put("bbt", beta_cols * np.asarray(inputs['bb'], f32), KD)
    put("bo", inputs['bo'], KD)
    put("b1", inputs['b1'], KF)
    put("b2", inputs['b2'], KD)
    put("g1", inputs['g1'], KD)
    put("bn1", inputs['bn1'], KD)
    put("g2", inputs['g2'], KD)
    put("bn2", inputs['bn2'], KD)

    inv = 1.0 / (10000.0 ** (np.arange(0, HD, 2, dtype=np.float64) / HD))
    freqs = np.arange(S, dtype=np.float64)[None, :] * inv[:, None]
    cos64 = np.repeat(np.cos(freqs), 2, axis=0).astype(f32)
    sin64 = np.repeat(np.sin(freqs), 2, axis=0).astype(f32)
    cos_t = np.ascontiguousarray(np.concatenate([cos64, cos64], axis=0).astype(f16))
    sin_t = np.ascontiguousarray(np.concatenate([sin64, sin64], axis=0).astype(f16))

    R64 = np.zeros((HD, HD), f32)
    for i in range(HD // 2):
        R64[2 * i, 2 * i + 1] = -1.0
        R64[2 * i + 1, 2 * i] = 1.0
    R128 = np.zeros((128, 128), f32)
    R128[:64, :64] = R64
    R128[64:, 64:] = R64

    def wprep(w, kblk, dout):
        wt = np.asarray(w, f32).T
        return np.ascontiguousarray(wt.reshape(kblk, 128, dout).astype(f16))

    shared = {
        'wet': wprep(inputs['We'], KE, D),
        'wqt': wprep(inputs['Wq'], KD, D),
        'wkt': wprep(inputs['Wk'], KD, D),
        'wvt': wprep(inputs['Wv'], KD, D),
        'wot': wprep(inputs['Wo'], KD, D),
        'w1t': wprep(inputs['W1'], KD, DF),
        'w2t': wprep(inputs['W2'], KF, D),
        'bias_cols': bias_cols,
        'bv_row': np.ascontiguousarray(np.asarray(inputs['bv'], f32).reshape(1, D)),
        'cos_t': cos_t,
        'sin_t': sin_t,
        'r128t': np.ascontiguousarray(R128.T.astype(f16)),
        'ones_t': np.ones((128, 128), f16),
    }
    if with_beta:
        shared['wgt'] = wprep(inputs['Wg'], KG, D)
        shared['wbt'] = np.ascontiguousarray(
            (np.asarray(inputs['Wb'], f32).T * beta_cols[None, :])
            .reshape(KD, 128, D).astype(f16))

    pros = np.asarray(inputs['pros'], f32)
    struct = np.asarray(inputs['structure'], f32) if with_beta else None
    in_maps = []
    for c in range(N_CORES):
        b0 = c * B_LOC
        m = dict(shared)
        m['pros_t'] = np.ascontiguousarray(
            pros[b0:b0 + B_LOC].reshape(T, E).T.astype(f16)).reshape(KE, 128, T)
        if with_beta:
            m['struct_t'] = np.ascontiguousarray(
                struct[b0:b0 + B_LOC].reshape(T, G).T.astype(f16)).reshape(KG, 128, T)
        in_maps.append(m)
    return in_maps


def kernel(**inputs):
    from concourse.bass_utils import run_bass_kernel_spmd

    with_beta = bool(np.any(np.asarray(inputs['beta']) != 0))
    nc = _build_module(with_beta=with_beta)
    in_maps = _prep_inputs(inputs, with_beta=with_beta)
    trace = bool(int(os.environ.get("BGC_TRACE", "0")))
    res = run_bass_kernel_spmd(
        nc, in_maps, core_ids=list(range(N_CORES)), trace=trace,
    )
    LAST_RESULT.clear()
    LAST_RESULT['exec_time_ns'] = res.exec_time_ns
    LAST_RESULT['mean_exec_time_ns'] = res.mean_exec_time_ns
    LAST_RESULT['trace'] = res.instructions_and_trace

    out = np.empty((B, S, D), np.float32)
    for c in range(N_CORES):
        o = res.results[c]['out_t']           # [KD, 128, T]
        out_T = o.reshape(D, T)
        out[c * B_LOC:(c + 1) * B_LOC] = out_T.T.reshape(B_LOC, S, D)

    keep = (~np.asarray(inputs['mask']))[..., None].astype(np.float32)
    return out * keep


# revision 18
# speedup vs baseline: 1.0436x; 1.0436x over previous
"""Trainium2 Bass kernel for nn_BGCEncoder (transformer encoder block).

Data-parallel over batch: 16 batch elements / 8 cores = 2 per core.
Activations are feature-major [feat, tokens] on-chip so every matmul
contracts over the partition dim with zero on-device transposes.
All matmul operands are fp16 (fp32 PSUM accumulation); measured
end-to-end relative error ~1e-3.

Structure (per core, T = 2048 tokens):
  A:  x = gelu(WeT.T @ pros_T + be)                  [D, T] fp16
  B:  btl = Wb_s.T @ gelu(WgT.T @ struct_T + bg)     (beta folded into Wb;
      emitted ONLY when beta != 0 — for this model beta == 0 so the whole
      structure branch vanishes and btl == 0)
  V:  v3[t, h, 0:64] = x-as-lhsT @ WvT + bv ; v3[t, h, 64] = 1  (ones col
      gives the softmax denominator through the ctx matmul)
  C+D fused per (head-pair hp, batch b):
      q' = rope(Wq[hp] @ x) (+btl) ; k = rope(Wk[hp] @ x)  [128, 1024]
      per qt chunk (512): per kt block (128): pair-scores psum [128,1024]
      (two K=64 matmuls at row bases 0/64, concurrent), one Exp -> fp16,
      two ctx matmuls accumulate [65, 512] psums (row 64 = denominator).
      ctx + denom copied out unnormalized (DVE).
  Post-D: one batched reciprocal over all 32 denominator rows, then
      per row: K=1 broadcast matmul + DVE multiply to normalize ctx.
  Wo + residual + LN1 ; FFN (gelu) ; + residual LN2  (LN stats via
  ones-matmuls; rstd = Exp(-0.5*Ln(var+eps)) on ACT; row broadcasts via
  K=1 matmuls at base-0)
"""

import os
import numpy as np

B, S, E, G, D, H = 16, 1024, 1280, 3072, 512, 8
HD = D // H            # 64
EPS = 1e-5
N_CORES = 8
B_LOC = B // N_CORES   # 2
T = B_LOC * S          # 2048
KE, KG, KD = E // 128, G // 128, D // 128   # 10, 24, 4
DF = 2 * D             # 1024
KF = DF // 128         # 8
TC = 512               # token chunk (tail phases, attention qt)
NT = T // TC           # 4
TB = 1024              # big token chunk (projection phases)
NTB_BIG = T // TB      # 2
NTB = T // 128         # 16 token blocks (for v)

_BOFF = {}
_off = 0
for _name, _n in [("be", KD), ("bg", KD), ("bq", KD), ("bk", KD), ("bbt", KD),
                  ("bo", KD), ("b1", KF), ("b2", KD), ("g1", KD), ("bn1", KD),
                  ("g2", KD), ("bn2", KD)]:
    _BOFF[_name] = _off
    _off += _n
NBIAS = _off

LAST_RESULT = {}


def _build_module(sim_gelu=False, with_beta=True):
    import concourse.bass as bass
    from concourse import bacc
    import concourse.mybir as mybir
    from concourse.tile import TileContext

    F32 = mybir.dt.float32
    F16 = mybir.dt.float16
    AF = mybir.ActivationFunctionType
    GELU = AF.Sigmoid if sim_gelu else AF.Gelu
    MUL = mybir.AluOpType.mult
    ADD = mybir.AluOpType.add
    SUB = mybir.AluOpType.subtract

    nc = bacc.Bacc("TRN2", target_bir_lowering=False)

    # ---- DRAM tensors ----
    pros_d = nc.dram_tensor("pros_t", [KE, 128, T], F16, kind="ExternalInput")
    wet_d = nc.dram_tensor("wet", [KE, 128, D], F16, kind="ExternalInput")
    if with_beta:
        struct_d = nc.dram_tensor("struct_t", [KG, 128, T], F16, kind="ExternalInput")
        wgt_d = nc.dram_tensor("wgt", [KG, 128, D], F16, kind="ExternalInput")
        wbt_d = nc.dram_tensor("wbt", [KD, 128, D], F16, kind="ExternalInput")
    wqt_d = nc.dram_tensor("wqt", [KD, 128, D], F16, kind="ExternalInput")
    wkt_d = nc.dram_tensor("wkt", [KD, 128, D], F16, kind="ExternalInput")
    wvt_d = nc.dram_tensor("wvt", [KD, 128, D], F16, kind="ExternalInput")
    wot_d = nc.dram_tensor("wot", [KD, 128, D], F16, kind="ExternalInput")
    w1t_d = nc.dram_tensor("w1t", [KD, 128, DF], F16, kind="ExternalInput")
    w2t_d = nc.dram_tensor("w2t", [KF, 128, D], F16, kind="ExternalInput")
    bias_d = nc.dram_tensor("bias_cols", [128, NBIAS], F32, kind="ExternalInput")
    bv_d = nc.dram_tensor("bv_row", [1, D], F32, kind="ExternalInput")
    cos_d = nc.dram_tensor("cos_t", [128, S], F16, kind="ExternalInput")
    sin_d = nc.dram_tensor("sin_t", [128, S], F16, kind="ExternalInput")
    r128_d = nc.dram_tensor("r128t", [128, 128], F16, kind="ExternalInput")
    ones_d = nc.dram_tensor("ones_t", [128, 128], F16, kind="ExternalInput")
    out_d = nc.dram_tensor("out_t", [KD, 128, T], F32, kind="ExternalOutput")

    with TileContext(nc) as tc, nc.allow_low_precision(
            reason="fp16 matmul operands by design; fp32 accumulation in PSUM"):
        with (
            tc.tile_pool(name="const", bufs=1) as constp,
            tc.tile_pool(name="big", bufs=4) as bigp,
        ):
            # ---- constants ----
            bias_sb = constp.tile([128, NBIAS], F32, tag="bias")
            nc.sync.dma_start(out=bias_sb, in_=bias_d.ap())
            cos_sb = constp.tile([128, S], F16, tag="cos")
            sin_sb = constp.tile([128, S], F16, tag="sin")
            nc.sync.dma_start(out=cos_sb, in_=cos_d.ap())
            nc.sync.dma_start(out=sin_sb, in_=sin_d.ap())
            r128_sb = constp.tile([128, 128], F16, tag="r128")
            nc.sync.dma_start(out=r128_sb, in_=r128_d.ap())
            bv_bc = constp.tile([128, D], F32, tag="bvbc")
            nc.gpsimd.dma_start(out=bv_bc, in_=bv_d.ap()[0:1, :].to_broadcast((128, D)))
            ones_col = constp.tile([128, 1], F16, tag="ones_col")
            nc.sync.dma_start(out=ones_col, in_=ones_d.ap()[:, 0:1])
            ones128 = constp.tile([128, 128], F16, tag="ones128")
            nc.sync.dma_start(out=ones128, in_=ones_d.ap())
            eps_sb = constp.tile([128, 1], F32, tag="eps")
            nc.vector.memset(eps_sb, EPS)

            def bcol(name, blk):
                o = _BOFF[name] + blk
                return bias_sb[:, o:o + 1]

            x_sb = bigp.tile([128, KD, T], F16, tag="slab", name="x")

            # ============ phase A: x = gelu(We @ pros + be) ============
            with (
                tc.tile_pool(name="pha", bufs=3) as pha,
                tc.tile_pool(name="phaw", bufs=1) as phaw,
                tc.tile_pool(name="psA", bufs=4, space="PSUM") as psA,
            ):
                wet_sb = phaw.tile([128, KE, D], F16, tag="wet")
                nc.sync.dma_start(out=wet_sb, in_=wet_d.ap().rearrange("k p d -> p k d"))
                for i in range(NT):
                    ts = slice(i * TC, (i + 1) * TC)
                    ps = [psA.tile([128, TC], F32, tag="mm", name=f"psa{_k}")
                          for _k in range(KD)]
                    for kc in range(2):
                        pr = pha.tile([128, 5, TC], F16, tag="pros")
                        nc.sync.dma_start(
                            out=pr,
                            in_=pros_d.ap()[kc * 5:(kc + 1) * 5, :, ts]
                            .rearrange("k p t -> p k t"))
                        for kd in range(KD):
                            for k5 in range(5):
                                k = kc * 5 + k5
                                nc.tensor.matmul(
                                    ps[kd],
                                    wet_sb[:, k, kd * 128:(kd + 1) * 128],
                                    pr[:, k5, :],
                                    start=(k == 0), stop=(k == KE - 1))
                    for kd in range(KD):
                        nc.scalar.activation(
                            out=x_sb[:, kd, ts], in_=ps[kd],
                            func=GELU, bias=bcol("be", kd), scale=1.0)

            # ============ phase B (only when beta != 0) ============
            btl_sb = None
            if with_beta:
                btl_sb = bigp.tile([128, KD, T], F16, tag="slab", name="btl")
                with (
                    tc.tile_pool(name="phb", bufs=2) as phb,
                    tc.tile_pool(name="phbw", bufs=1) as phbw,
                    tc.tile_pool(name="psB", bufs=4, space="PSUM") as psB,
                ):
                    wgt_sb = phbw.tile([128, KG, D], F16, tag="wgt")
                    nc.sync.dma_start(out=wgt_sb,
                                      in_=wgt_d.ap().rearrange("k p d -> p k d"))
                    wbt_sb = phbw.tile([128, KD, D], F16, tag="wbt")
                    nc.sync.dma_start(out=wbt_sb,
                                      in_=wbt_d.ap().rearrange("k p d -> p k d"))
                    for i in range(NT):
                        ts = slice(i * TC, (i + 1) * TC)
                        ps = [psB.tile([128, TC], F32, tag="mm", name=f"psb{_k}")
                              for _k in range(KD)]
                        for kc in range(4):
                            sc = phb.tile([128, 6, TC], F16, tag="struct")
                            nc.sync.dma_start(
                                out=sc,
                                in_=struct_d.ap()[kc * 6:(kc + 1) * 6, :, ts]
                                .rearrange("k p t -> p k t"))
                            for kd in range(KD):
                                for k6 in range(6):
                                    k = kc * 6 + k6
                                    nc.tensor.matmul(
                                        ps[kd],
                                        wgt_sb[:, k, kd * 128:(kd + 1) * 128],
                                        sc[:, k6, :],
                                        start=(k == 0), stop=(k == KG - 1))
                        stc = phb.tile([128, KD, TC], F16, tag="st")
                        for kd in range(KD):
                            nc.scalar.activation(
                                out=stc[:, kd, :], in_=ps[kd],
                                func=GELU, bias=bcol("bg", kd), scale=1.0)
                        for kd in range(KD):
                            pb = psB.tile([128, TC], F32, tag="mm")
                            for k in range(KD):
                                nc.tensor.matmul(
                                    pb, wbt_sb[:, k, kd * 128:(kd + 1) * 128],
                                    stc[:, k, :],
                                    start=(k == 0), stop=(k == KD - 1))
                            nc.scalar.activation(
                                out=btl_sb[:, kd, ts], in_=pb,
                                func=AF.Identity, bias=bcol("bbt", kd), scale=1.0)

            # ============ phase V: v3 (token-major v + ones column) ============
            with (
                tc.tile_pool(name="v3pool", bufs=1) as v3p,
            ):
                v3_sb = v3p.tile([128, NTB, H, HD + 1], F16, tag="v3")
                nc.sync.dma_start(
                    out=v3_sb[:, :, :, HD:HD + 1],
                    in_=ones_d.ap().rearrange("p (a b) -> p a b", b=8)[:, :, :, None])
                with (
                    tc.tile_pool(name="phvw", bufs=1) as phvw,
                    tc.tile_pool(name="psVp", bufs=4, space="PSUM") as psVp,
                ):
                    wv_sb = phvw.tile([128, KD, D], F16, tag="wv")
                    nc.sync.dma_start(out=wv_sb,
                                      in_=wvt_d.ap().rearrange("k p d -> p k d"))
                    for tb in range(NTB):
                        pv = psVp.tile([128, D], F32, tag="mm")
                        for k in range(KD):
                            nc.tensor.matmul(
                                pv, x_sb[:, k, tb * 128:(tb + 1) * 128],
                                wv_sb[:, k, :],
                                start=(k == 0), stop=(k == KD - 1))
                        nc.vector.tensor_tensor(
                            v3_sb[:, tb, :, 0:HD], pv, bv_bc, ADD)

                # ======== fused C+D: per head pair, per batch ========
                qp_sb = bigp.tile([128, KD, T], F16, tag="slab", name="qp")
                kr_sb = bigp.tile([128, KD, T], F16, tag="slab", name="kr")
                ctx_sb = bigp.tile([128, KD, T], F16, tag="slab", name="ctx")
                with (
                    tc.tile_pool(name="phc", bufs=3) as phc,
                    tc.tile_pool(name="phcw", bufs=1) as phcw,
                    tc.tile_pool(name="phd", bufs=6) as phd,
                    tc.tile_pool(name="dnp", bufs=2) as dnp,
                    tc.tile_pool(name="psC", bufs=2, space="PSUM") as psC,
                    tc.tile_pool(name="psS", bufs=2, space="PSUM") as psS,
                    tc.tile_pool(name="psX", bufs=2, space="PSUM") as psX,
                ):
                    wq_sb = phcw.tile([128, KD, D], F16, tag="wq")
                    nc.sync.dma_start(out=wq_sb,
                                      in_=wqt_d.ap().rearrange("k p d -> p k d"))
                    wk_sb = phcw.tile([128, KD, D], F16, tag="wk")
                    nc.sync.dma_start(out=wk_sb,
                                      in_=wkt_d.ap().rearrange("k p d -> p k d"))

                    def proj_rope(w_sb, bname, dst, add_btl, hp, i):
                        # one batch (TB=1024 tokens): matmuls in 512 halves,
                        # DVE rope over the full 1024-wide tiles
                        qt = phc.tile([128, TB], F16, tag="qtmp")
                        for half in range(2):
                            hs = slice(i * TB + half * TC, i * TB + (half + 1) * TC)
                            pq = psC.tile([128, TC], F32, tag="pq")
                            for k in range(KD):
                                nc.tensor.matmul(
                                    pq, w_sb[:, k, hp * 128:(hp + 1) * 128],
                                    x_sb[:, k, hs],
                                    start=(k == 0), stop=(k == KD - 1))
                            nc.vector.tensor_scalar(
                                out=qt[:, half * TC:(half + 1) * TC], in0=pq,
                                scalar1=bcol(bname, hp), scalar2=None, op0=ADD)
                        t2 = phc.tile([128, TB], F16, tag="rt2")
                        for half in range(2):
                            prot = psC.tile([128, TC], F32, tag="pq")
                            nc.tensor.matmul(prot, r128_sb,
                                             qt[:, half * TC:(half + 1) * TC],
                                             start=True, stop=True)
                            nc.vector.tensor_tensor(
                                t2[:, half * TC:(half + 1) * TC], prot,
                                sin_sb[:, half * TC:(half + 1) * TC], MUL)
                        t1 = phc.tile([128, TB], F16, tag="rt1")
                        nc.vector.tensor_tensor(t1, qt, cos_sb, MUL)
                        ts = slice(i * TB, (i + 1) * TB)
                        dslice = dst[:, hp, ts]
                        if add_btl:
                            nc.vector.tensor_tensor(t1, t1, t2, ADD)
                            nc.vector.tensor_tensor(
                                dslice, t1, btl_sb[:, hp, ts], ADD)
                        else:
                            nc.vector.tensor_tensor(dslice, t1, t2, ADD)

                    scale = float(1.0 / np.sqrt(HD))
                    NQ = S // TC   # qt chunks per batch (2)
                    NJ = S // 128  # kt blocks per batch (8)
                    for hp in range(KD):
                        for b in range(B_LOC):
                            proj_rope(wq_sb, "bq", qp_sb, with_beta, hp, b)
                            proj_rope(wk_sb, "bk", kr_sb, False, hp, b)
                            # 4 denominator rows (qi x hh) packed at legal bases
                            dn_pack = dnp.tile([128, TC], F32, tag="dn")
                            nc.vector.memset(dn_pack, 1.0)
                            for qi in range(NQ):
                                qcol = b * S + qi * TC
                                c0 = psX.tile([HD + 1, TC], F32, tag="ctx", name="c0")
                                c1 = psX.tile([HD + 1, TC], F32, tag="ctx", name="c1")
                                cpair = (c0, c1)
                                for j in range(NJ):
                                    kcol = b * S + j * 128
                                    sp = psS.tile([128, TB], F32, tag="sc")
                                    for hh in range(2):
                                        r0 = hh * 64
                                        nc.tensor.matmul(
                                            sp[:, hh * TC:(hh + 1) * TC],
                                            kr_sb[r0:r0 + 64, hp, kcol:kcol + 128],
                                            qp_sb[r0:r0 + 64, hp, qcol:qcol + TC],
                                            start=True, stop=True)
                                    ee = phd.tile([128, TB], F16, tag="exp")
                                    nc.scalar.activation(out=ee, in_=sp, func=AF.Exp,
                                                         scale=scale)
                                    for hh in range(2):
                                        nc.tensor.matmul(
                                            cpair[hh],
                                            v3_sb[:, b * 8 + j, hp * 2 + hh, :],
                                            ee[:, hh * TC:(hh + 1) * TC],
                                            start=(j == 0), stop=(j == NJ - 1))
                                for hh in range(2):
                                    r0 = hh * 64
                                    base = 32 * (qi * 2 + hh)
                                    nc.vector.tensor_copy(
                                        out=ctx_sb[r0:r0 + 64, hp, qcol:qcol + TC],
                                        in_=cpair[hh][0:HD, :])
                                    nc.vector.tensor_copy(
                                        out=dn_pack[base:base + 1, :],
                                        in_=cpair[hh][HD:HD + 1, :])
                            # one reciprocal covers the 4 rows; rows already
                            # sit at legal bases for the K=1 broadcast matmul
                            dinv_pk = dnp.tile([128, TC], F16, tag="dinv")
                            nc.vector.reciprocal(out=dinv_pk, in_=dn_pack)
                            for qi in range(NQ):
                                qcol = b * S + qi * TC
                                for hh in range(2):
                                    r0 = hh * 64
                                    base = 32 * (qi * 2 + hh)
                                    pbc = psS.tile([128, TB], F32, tag="sc")
                                    nc.tensor.matmul(
                                        pbc[0:64, 0:TC],
                                        ones128[base:base + 1, 0:64],
                                        dinv_pk[base:base + 1, :],
                                        start=True, stop=True,
                                        tile_position=(base, 0))
                                    cslice = ctx_sb[r0:r0 + 64, hp, qcol:qcol + TC]
                                    nc.vector.tensor_tensor(
                                        cslice, cslice, pbc[0:64, 0:TC], MUL)

            # ============ Wo + residual + LN1 ============
            h_sb = bigp.tile([128, KD, T], F16, tag="slab", name="h")

            def layernorm(i, z, gname, bname, dst, lnp, psbcln):
                ps1 = psbcln.tile([1, TC], F32, tag="s1")
                ps2 = psbcln.tile([1, TC], F32, tag="s2")
                sq = lnp.tile([128, KD, TC], F16, tag="sq")
                for kd in range(KD):
                    nc.vector.tensor_tensor(sq[:, kd, :], z[:, kd, :], z[:, kd, :], MUL)
                for kd in range(KD):
                    nc.tensor.matmul(ps1, ones_col, z[:, kd, :],
                                     start=(kd == 0), stop=(kd == KD - 1))
                for kd in range(KD):
                    nc.tensor.matmul(ps2, ones_col, sq[:, kd, :],
                                     start=(kd == 0), stop=(kd == KD - 1))
                mrow = lnp.tile([1, TC], F32, tag="mrow")
                nc.vector.tensor_scalar_mul(mrow, ps1, 1.0 / D)
                vrow = lnp.tile([1, TC], F32, tag="vrow")
                nc.vector.tensor_scalar_mul(vrow, ps2, 1.0 / D)
                m2 = lnp.tile([1, TC], F32, tag="m2row")
                nc.vector.tensor_tensor(m2, mrow, mrow, MUL)
                nc.vector.tensor_tensor(vrow, vrow, m2, SUB)
                # rstd = exp(-0.5 * ln(var + eps)) on ACT (avoids slow DVE recip)
                lrow = lnp.tile([1, TC], F32, tag="lrow")
                nc.scalar.activation(out=lrow, in_=vrow, func=AF.Ln,
                                     bias=eps_sb[0:1, :], scale=1.0)
                rstd = lnp.tile([1, TC], F16, tag="rstd")
                nc.scalar.activation(out=rstd, in_=lrow, func=AF.Exp, scale=-0.5)
                sh = lnp.tile([1, TC], F16, tag="shrow")
                nc.vector.tensor_tensor(sh, mrow, rstd, MUL)
                nc.vector.tensor_scalar_mul(sh, sh, -1.0)
                psc = psbcln.tile([128, TC], F32, tag="scbc")
                nc.tensor.matmul(psc, ones128[0:1, :], rstd, start=True, stop=True)
                psh = psbcln.tile([128, TC], F32, tag="shbc")
                nc.tensor.matmul(psh, ones128[0:1, :], sh, start=True, stop=True)
                for kd in range(KD):
                    u = lnp.tile([128, TC], F32, tag="u")
                    nc.vector.tensor_tensor(u, z[:, kd, :], psc, MUL)
                    nc.vector.tensor_tensor(u, u, psh, ADD)
                    nc.scalar.activation(
                        out=dst[:, kd, :] if dst.shape[-1] == TC
                        else dst[:, kd, i * TC:(i + 1) * TC],
                        in_=u, func=AF.Identity,
                        bias=bcol(bname, kd), scale=bcol(gname, kd))

            with (
                tc.tile_pool(name="lnp", bufs=4) as lnp,
                tc.tile_pool(name="phow", bufs=1) as phow,
                tc.tile_pool(name="psO", bufs=4, space="PSUM") as psO,
                tc.tile_pool(name="psbcln", bufs=1, space="PSUM") as psbcln,
            ):
                wot_sb = phow.tile([128, KD, D], F16, tag="wot")
                nc.sync.dma_start(out=wot_sb, in_=wot_d.ap().rearrange("k p d -> p k d"))
                for i in range(NT):
                    ts = slice(i * TC, (i + 1) * TC)
                    z = lnp.tile([128, KD, TC], F16, tag="z")
                    for kd in range(KD):
                        po = psO.tile([128, TC], F32, tag="mm")
                        for k in range(KD):
                            nc.tensor.matmul(
                                po, wot_sb[:, k, kd * 128:(kd + 1) * 128],
                                ctx_sb[:, k, ts],
                                start=(k == 0), stop=(k == KD - 1))
                        za = lnp.tile([128, TC], F32, tag="za")
                        nc.scalar.activation(out=za, in_=po, func=AF.Identity,
                                             bias=bcol("bo", kd), scale=1.0)
                        nc.vector.tensor_tensor(z[:, kd, :], za, x_sb[:, kd, ts], ADD)
                    layernorm(i, z, "g1", "bn1", h_sb, lnp, psbcln)

            # ============ FFN + LN2 ============
            ff1a = bigp.tile([128, KD, T], F16, tag="slab", name="ff1a")
            ff1b = bigp.tile([128, KD, T], F16, tag="slab", name="ff1b")
            with (
                tc.tile_pool(name="lnp2", bufs=4) as lnp2,
                tc.tile_pool(name="phfw", bufs=1) as phfw,
                tc.tile_pool(name="outp", bufs=2) as outp,
            ):
                w1_sb = phfw.tile([128, KD, DF], F16, tag="w1")
                nc.sync.dma_start(out=w1_sb, in_=w1t_d.ap().rearrange("k p d -> p k d"))
                w2_sb = phfw.tile([128, KF, D], F16, tag="w2")
                nc.sync.dma_start(out=w2_sb, in_=w2t_d.ap().rearrange("k p d -> p k d"))
                with tc.tile_pool(name="psF1", bufs=3, space="PSUM") as psF1:
                    for i in range(NT):
                        ts = slice(i * TC, (i + 1) * TC)
                        for kf in range(KF):
                            pf = psF1.tile([128, TC], F32, tag="mm")
                            for k in range(KD):
                                nc.tensor.matmul(
                                    pf, w1_sb[:, k, kf * 128:(kf + 1) * 128],
                                    h_sb[:, k, ts],
                                    start=(k == 0), stop=(k == KD - 1))
                            dstf = ff1a if kf < KD else ff1b
                            nc.scalar.activation(
                                out=dstf[:, kf % KD, ts], in_=pf,
                                func=GELU, bias=bcol("b1", kf), scale=1.0)
                with (
                    tc.tile_pool(name="psF2", bufs=4, space="PSUM") as psF2,
                    tc.tile_pool(name="psbcln2", bufs=1, space="PSUM") as psbcln2,
                ):
                  for i in range(NT):
                    ts = slice(i * TC, (i + 1) * TC)
                    z2 = lnp2.tile([128, KD, TC], F16, tag="z")
                    for kd in range(KD):
                        p2 = psF2.tile([128, TC], F32, tag="mm2")
                        for k in range(KF):
                            src = ff1a if k < KD else ff1b
                            nc.tensor.matmul(
                                p2, w2_sb[:, k, kd * 128:(kd + 1) * 128],
                                src[:, k % KD, ts],
                                start=(k == 0), stop=(k == KF - 1))
                        za = lnp2.tile([128, TC], F32, tag="za")
                        nc.scalar.activation(out=za, in_=p2, func=AF.Identity,
                                             bias=bcol("b2", kd), scale=1.0)
                        nc.vector.tensor_tensor(z2[:, kd, :], za, h_sb[:, kd, ts], ADD)
                    oc = outp.tile([128, KD, TC], F32, tag="oc")
                    layernorm(i, z2, "g2", "bn2", oc, lnp2, psbcln2)
                    for kd in range(KD):
                        nc.sync.dma_start(out=out_d.ap()[kd, :, ts], in_=oc[:, kd, :])

    nc.finalize()
    return nc


def _prep_inputs(inputs, with_beta=True):
    f32 = np.float32
    f16 = np.float16

    def col4(vec, nblk):
        return np.ascontiguousarray(np.asarray(vec, f32).reshape(nblk, 128).T)

    beta_cols = np.repeat(np.asarray(inputs['beta'], f32), HD)  # [D]

    bias_cols = np.zeros((128, NBIAS), f32)
    def put(name, vec, nblk):
        bias_cols[:, _BOFF[name]:_BOFF[name] + nblk] = col4(vec, nblk)
    put("be", inputs['be'], KD)
    put("bg", inputs['bg'], KD)
    put("bq", inputs['bq'], KD)
    put("bk", inputs['bk'], KD)
    put("bbt", beta_cols * np.asarray(inputs['bb'], f32), KD)
    put("bo", inputs['bo'], KD)
    put("b1", inputs['b1'], KF)
    put("b2", inputs['b2'], KD)
    put("g1", inputs['g1'], KD)
    put("bn1", inputs['bn1'], KD)
    put("g2", inputs['g2'], KD)
    put("bn2", inputs['bn2'], KD)

    inv = 1.0 / (10000.0 ** (np.arange(0, HD, 2, dtype=np.float64) / HD))
    freqs = np.arange(S, dtype=np.float64)[None, :] * inv[:, None]
    cos64 = np.repeat(np.cos(freqs), 2, axis=0).astype(f32)
    sin64 = np.repeat(np.sin(freqs), 2, axis=0).astype(f32)
    cos_t = np.ascontiguousarray(np.concatenate([cos64, cos64], axis=0).astype(f16))
    sin_t = np.ascontiguousarray(np.concatenate([sin64, sin64], axis=0).astype(f16))

    R64 = np.zeros((HD, HD), f32)
    for i in range(HD // 2):
        R64[2 * i, 2 * i + 1] = -1.0
        R64[2 * i + 1, 2 * i] = 1.0
    R128 = np.zeros((128, 128), f32)
    R128[:64, :64] = R64
    R128[64:, 64:] = R64

    def wprep(w, kblk, dout):
        wt = np.asarray(w, f32).T
        return np.ascontiguousarray(wt.reshape(kblk, 128, dout).astype(f16))

    shared = {
        'wet': wprep(inputs['We'], KE, D),
        'wqt': wprep(inputs['Wq'], KD, D),
        'wkt': wprep(inputs['Wk'], KD, D),
        'wvt': wprep(inputs['Wv'], KD, D),
        'wot': wprep(inputs['Wo'], KD, D),
        'w1t': wprep(inputs['W1'], KD, DF),
        'w2t': wprep(inputs['W2'], KF, D),
        'bias_cols': bias_cols,
        'bv_row': np.ascontiguousarray(np.asarray(inputs['bv'], f32).reshape(1, D)),
        'cos_t': cos_t,
        'sin_t': sin_t,
        'r128t': np.ascontiguousarray(R128.T.astype(f16)),
        'ones_t': np.ones((128, 128), f16),
    }
    if with_beta:
        shared['wgt'] = wprep(inputs['Wg'], KG, D)
        shared['wbt'] = np.ascontiguousarray(
            (np.asarray(inputs['Wb'], f32).T * beta_cols[None, :])
            .reshape(KD, 128, D).astype(f16))

    pros = np.asarray(inputs['pros'], f32)
    struct = np.asarray(inputs['structure'], f32) if with_beta else None
    in_maps = []
    for c in range(N_CORES):
        b0 = c * B_LOC
        m = dict(shared)
        m['pros_t'] = np.ascontiguousarray(
            pros[b0:b0 + B_LOC].reshape(T, E).T.astype(f16)).reshape(KE, 128, T)
        if with_beta:
            m['struct_t'] = np.ascontiguousarray(
                struct[b0:b0 + B_LOC].reshape(T, G).T.astype(f16)).reshape(KG, 128, T)
        in_maps.append(m)
    return in_maps


def kernel(**inputs):
    from concourse.bass_utils import run_bass_kernel_spmd

    with_beta = bool(np.any(np.asarray(inputs['beta']) != 0))
    nc = _build_module(with_beta=with_beta)
    in_maps = _prep_inputs(inputs, with_beta=with_beta)
    trace = bool(int(os.environ.get("BGC_TRACE", "0")))
    res = run_bass_kernel_spmd(
        nc, in_maps, core_ids=list(range(N_CORES)), trace=trace,
    )
    LAST_RESULT.clear()
    LAST_RESULT['exec_time_ns'] = res.exec_time_ns
    LAST_RESULT['mean_exec_time_ns'] = res.mean_exec_time_ns
    LAST_RESULT['trace'] = res.instructions_and_trace

    out = np.empty((B, S, D), np.float32)
    for c in range(N_CORES):
        o = res.results[c]['out_t']           # [KD, 128, T]
        out_T = o.reshape(D, T)
        out[c * B_LOC:(c + 1) * B_LOC] = out_T.T.reshape(B_LOC, S, D)

    keep = (~np.asarray(inputs['mask']))[..., None].astype(np.float32)
    return out * keep
